# revision 6
# baseline (speedup 1.0000x reference)
"""Neural ODE layer (3-layer tanh MLP dynamics, RK4, 10 steps) on 8 trn2 cores.

Strategy: data-parallel over batch (8192/8 = 1024 rows per core), weights
replicated (no cross-device communication). Inside each core the batch is
split into 2 chunks of 512 columns, both SBUF-resident. All activations
live in SBUF transposed ([hid on partitions, batch free]) so every matmul
is out^T = W^T @ x^T with the weight slice stationary and the activation
moving -- the output lands in exactly the layout the next layer needs, so
the whole matmul chain runs without a single transpose.

fp8 mode (default): matmul operands are fp8-e4m3 with perf_mode=DoubleRow
(2 fp8 weights per PE cell -> 2 contraction rows/cycle). Weights are
pre-scaled by SW=2048 on the host so U(-1/32,1/32) lands in e4m3's normal
range; the 1/SW descale is folded into the PSUM-drain scale of the
activation/combine ops. Activations (tanh outputs, |x|<=1, and the state
h, |h|~N(0,1)) are cast to e4m3 unscaled -- values below the 2^-6 normal
floor contribute negligibly to the 1024-term dot products. The
integration state h and the RK4 accumulator stay fp32.

paired mode (default): the two 512-column chunks' matmuls are interleaved
at the innermost level so consecutive matmuls share the same stationary
weight slice (amortizes LDWEIGHTS across 1024 moving columns), and the
PSUM drains of both chunks overlap the next m-tile's matmuls.

The t-input is folded into per-eval bias vectors
(concat(h,t) @ W1 == h @ W1[:-1] + t*W1[-1]), and the RK4 combine
(h + c*k accumulation) is fused into the PSUM-drain ops on ACT/DVE.

Built as bacc.Bacc and finished with nc.compile(): that pass splits
multi-semaphore waits into EventSemaphore instructions (TRN2 allows one
sync wait per instruction) -- without it walrus codegen rejects any
cross-engine Tile kernel.
"""

import sys

sys.path.insert(0, "/opt/trn_rl_repo")

import numpy as np
import ml_dtypes
from contextlib import ExitStack

import concourse.bacc as bacc
import concourse.tile as tile
from concourse import mybir
from concourse.bass_utils import run_bass_kernel_spmd

HID = 1024
BATCH = 8192
N_CORES = 8
CORE_BATCH = BATCH // N_CORES  # 1024
DT = 0.1
STEPS = 10
P = 128
KT = HID // P  # 8 contraction tiles
MT = HID // P  # 8 output tiles
NCHUNK = 512   # batch columns per chunk (= one fp32 PSUM bank)
CHUNKS = CORE_BATCH // NCHUNK  # 2
SW = 2048.0    # fp8 weight pre-scale: U(-1/32,1/32) -> +-64 (e4m3 normal)

F32 = mybir.dt.float32
FP16 = mybir.dt.float16
FP8 = mybir.dt.float8e4
AF = mybir.ActivationFunctionType
ALU = mybir.AluOpType
DR = mybir.MatmulPerfMode.DoubleRow

# RK4: h' = h + dt/6*(k1 + 2k2 + 2k3 + k4)
ACC_W = [DT / 6, DT / 3, DT / 3, DT / 6]   # weight of k_e in the combine
STEP_C = [DT / 2, DT / 2, DT]              # h_tmp = h + c*k_e  (evals 0..2)
T_OFF = [0, 1, 1, 2]                       # t index offset (in dt/2 units)


def build_nc(steps=STEPS, chunks=CHUNKS, reps=1, mode="fp8", paired=True):
    fp8 = mode == "fp8"
    ACT_DT = FP8 if fp8 else FP16   # matmul operand dtype
    wdiv = SW if fp8 else 1.0       # descale folded into PSUM drains

    nc = bacc.Bacc("TRN2", target_bir_lowering=False, debug=False)

    h_in = nc.dram_tensor("h", [CORE_BATCH, HID], F32, kind="ExternalInput").ap()
    W1 = nc.dram_tensor("W1", [HID, HID], ACT_DT, kind="ExternalInput").ap()
    w1row = nc.dram_tensor("w1row", [HID], F32, kind="ExternalInput").ap()
    b1 = nc.dram_tensor("b1", [HID], F32, kind="ExternalInput").ap()
    W2 = nc.dram_tensor("W2", [HID, HID], ACT_DT, kind="ExternalInput").ap()
    b2 = nc.dram_tensor("b2", [HID], F32, kind="ExternalInput").ap()
    W3 = nc.dram_tensor("W3", [HID, HID], ACT_DT, kind="ExternalInput").ap()
    b3 = nc.dram_tensor("b3", [HID], F32, kind="ExternalInput").ap()
    ident = nc.dram_tensor("ident", [P, P], F32, kind="ExternalInput").ap()
    out = nc.dram_tensor("out", [CORE_BATCH, HID], F32, kind="ExternalOutput").ap()

    n_t = 2 * steps + 1  # distinct t values on the dt/2 grid

    with tile.TileContext(nc) as tc, ExitStack() as ctx:
        pers = ctx.enter_context(tc.tile_pool(name="pers", bufs=1))
        stage_pool = ctx.enter_context(tc.tile_pool(name="stage", bufs=3))
        # paired mode: tags ps0/ps1 each get `bufs` ring slots -> 2*bufs banks
        psmm = ctx.enter_context(
            tc.tile_pool(name="psmm", bufs=3 if paired else 5, space="PSUM")
        )
        pstr = ctx.enter_context(tc.tile_pool(name="pstr", bufs=2, space="PSUM"))

        # weights: [p, k, m*P+j] = W[k*P+p, m*P+j]
        w1s = pers.tile([P, KT, HID], ACT_DT, tag="w1s")
        w2s = pers.tile([P, KT, HID], ACT_DT, tag="w2s")
        w3s = pers.tile([P, KT, HID], ACT_DT, tag="w3s")
        # activations, transposed: [p, m, b] = x[b, m*P+p]; one set per
        # 512-column batch chunk -- both chunks stay resident
        hT, hTb, acc, x0, x1 = [], [], [], [], []
        for c in range(chunks):
            hT_c = pers.tile([P, MT, NCHUNK], F32, tag=f"hT{c}", name=f"hT{c}")
            hTb_c = pers.tile([P, MT, NCHUNK], ACT_DT, tag=f"hTb{c}", name=f"hTb{c}")
            acc_c = pers.tile([P, MT, NCHUNK], F32, tag=f"acc{c}", name=f"acc{c}")
            x0_c = pers.tile([P, MT, NCHUNK], ACT_DT, tag=f"x0{c}", name=f"x0{c}")
            x1_c = pers.tile([P, MT, NCHUNK], ACT_DT, tag=f"x1{c}", name=f"x1{c}")
            hT.append(hT_c); hTb.append(hTb_c); acc.append(acc_c)
            x0.append(x0_c); x1.append(x1_c)
        idt = pers.tile([P, P], F32, tag="idt")
        # per-partition bias columns: [p, m] = v[m*P+p]
        w1r = pers.tile([P, MT], F32, tag="w1r")
        b1t = pers.tile([P, MT], F32, tag="b1t")
        b2t = pers.tile([P, MT], F32, tag="b2t")
        b3t = pers.tile([P, MT], F32, tag="b3t")
        b3dt = pers.tile([P, MT], F32, tag="b3dt")    # dt * b3
        b3h = pers.tile([P, MT], F32, tag="b3h")      # dt/2 * b3
        b1eff = pers.tile([P, MT, n_t], F32, tag="b1eff")  # b1 + t*W1[-1]

        dma = nc.sync.dma_start

        for ws, W in [(w1s, W1), (w2s, W2), (w3s, W3)]:
            for k in range(KT):
                dma(out=ws[:, k, :], in_=W[P * k : P * (k + 1), :])
        dma(out=idt[:], in_=ident)
        dma(out=w1r[:], in_=w1row.rearrange("(m p) -> p m", p=P))
        dma(out=b1t[:], in_=b1.rearrange("(m p) -> p m", p=P))
        dma(out=b2t[:], in_=b2.rearrange("(m p) -> p m", p=P))
        dma(out=b3t[:], in_=b3.rearrange("(m p) -> p m", p=P))

        nc.vector.tensor_scalar_mul(b3dt[:], b3t[:], DT)
        nc.vector.tensor_scalar_mul(b3h[:], b3t[:], DT / 2)
        for ti in range(n_t):
            nc.vector.scalar_tensor_tensor(
                b1eff[:, :, ti], w1r[:], ti * DT / 2, b1t[:], ALU.mult, ALU.add
            )

        def mm_chain(ps, ws, src, m):
            """psum[m] = sum_k ws[k,m]^T @ src[k] (DoubleRow pairs if fp8)."""
            if fp8:
                for kp in range(KT // 2):
                    nc.tensor.matmul(
                        ps[:],
                        ws[:, 2 * kp : 2 * kp + 2, P * m : P * (m + 1)],
                        src[:, 2 * kp : 2 * kp + 2, :],
                        start=(kp == 0),
                        stop=(kp == KT // 2 - 1),
                        perf_mode=DR,
                    )
            else:
                for k in range(KT):
                    nc.tensor.matmul(
                        ps[:],
                        ws[:, k, P * m : P * (m + 1)],
                        src[:, k, :],
                        start=(k == 0),
                        stop=(k == KT - 1),
                    )

        def layer_paired(srcs, ws, drains):
            """Both chunks' matmuls interleaved so consecutive matmuls
            share one stationary weight slice; drains overlap next m."""
            nch = len(srcs)
            for m in range(MT):
                pss = [
                    psmm.tile([P, NCHUNK], F32, tag=f"ps{c}", name=f"ps{c}")
                    for c in range(nch)
                ]
                if fp8:
                    for kp in range(KT // 2):
                        w_sl = ws[:, 2 * kp : 2 * kp + 2, P * m : P * (m + 1)]
                        for c in range(nch):
                            nc.tensor.matmul(
                                pss[c][:], w_sl,
                                srcs[c][:, 2 * kp : 2 * kp + 2, :],
                                start=(kp == 0), stop=(kp == KT // 2 - 1),
                                perf_mode=DR,
                            )
                else:
                    for k in range(KT):
                        w_sl = ws[:, k, P * m : P * (m + 1)]
                        for c in range(nch):
                            nc.tensor.matmul(
                                pss[c][:], w_sl, srcs[c][:, k, :],
                                start=(k == 0), stop=(k == KT - 1),
                            )
                for c in range(nch):
                    drains[c](pss[c], m)

        def layer(src, ws, drain):
            for m in range(MT):
                ps = psmm.tile([P, NCHUNK], F32, tag="ps")
                mm_chain(ps, ws, src, m)
                drain(ps, m)

        # ---- load all chunks, transposed via PE ----
        for c in range(chunks):
            rows0 = c * NCHUNK
            for bt in range(NCHUNK // P):
                stg = stage_pool.tile([P, HID], F32, tag="stg")
                dma(out=stg[:], in_=h_in[rows0 + P * bt : rows0 + P * (bt + 1), :])
                for j in range(MT):
                    pt = pstr.tile([P, P], F32, tag="pt")
                    nc.tensor.transpose(pt[:], stg[:, P * j : P * (j + 1)], idt[:])
                    nc.vector.tensor_copy(hT[c][:, j, P * bt : P * (bt + 1)], pt[:])
                    nc.vector.tensor_copy(hTb[c][:, j, P * bt : P * (bt + 1)], pt[:])

        # ---- RK4 steps ----
        def make_drains(ev, tidx, c):
            srcs = [hTb[c], x0[c], x1[c], x0[c]]
            d1s = [x0[c], x1[c], x0[c], x1[c]]
            d2s = [x1[c], x0[c], x1[c], x0[c]]

            def drain_tanh1(ps, m):
                nc.scalar.activation(
                    d1s[ev][:, m, :], ps[:], AF.Tanh,
                    bias=b1eff[:, m, tidx : tidx + 1], scale=1.0 / wdiv,
                )

            def drain_tanh2(ps, m):
                nc.scalar.activation(
                    d2s[ev][:, m, :], ps[:], AF.Tanh,
                    bias=b2t[:, m : m + 1], scale=1.0 / wdiv,
                )

            def drain_k(ps, m):
                # ps = wdiv*(k_e - b3) (b3 folded into the combines below)
                if ev == 0:
                    # acc = h + (dt/6)*ps1   (b3 terms folded at ev3)
                    nc.vector.scalar_tensor_tensor(
                        acc[c][:, m, :], ps[:], ACC_W[0] / wdiv, hT[c][:, m, :],
                        ALU.mult, ALU.add,
                    )
                elif ev == 3:
                    # hT = acc + (dt/6)*ps4 + dt*b3  -> new state
                    nc.scalar.activation(
                        hT[c][:, m, :], ps[:], AF.Identity,
                        bias=b3dt[:, m : m + 1], scale=ACC_W[3] / wdiv,
                    )
                    nc.vector.tensor_add(
                        hT[c][:, m, :], hT[c][:, m, :], acc[c][:, m, :]
                    )
                    nc.vector.tensor_copy(hTb[c][:, m, :], hT[c][:, m, :])
                else:
                    nc.vector.scalar_tensor_tensor(
                        acc[c][:, m, :], ps[:], ACC_W[ev] / wdiv, acc[c][:, m, :],
                        ALU.mult, ALU.add,
                    )
                if ev < 3:
                    # h_tmp = h + c*(ps + b3), into d1s[ev]'s buffer
                    # (free again: layer 2 has consumed it)
                    ht = d1s[ev]
                    cb = b3h if ev < 2 else b3dt
                    nc.scalar.activation(
                        ht[:, m, :], ps[:], AF.Identity,
                        bias=cb[:, m : m + 1], scale=STEP_C[ev] / wdiv,
                    )
                    nc.vector.tensor_add(
                        ht[:, m, :], ht[:, m, :], hT[c][:, m, :]
                    )

            return srcs, d1s, d2s, drain_tanh1, drain_tanh2, drain_k

        def steps_body():
          for st in range(steps):
              for ev in range(4):
                  tidx = 2 * st + T_OFF[ev]
                  plans = [make_drains(ev, tidx, c) for c in range(chunks)]
                  if paired:
                      layer_paired([p[0][ev] for p in plans], w1s,
                                   [p[3] for p in plans])
                      layer_paired([p[1][ev] for p in plans], w2s,
                                   [p[4] for p in plans])
                      layer_paired([p[2][ev] for p in plans], w3s,
                                   [p[5] for p in plans])
                  else:
                      # alternate chunks per layer: while chunk A's drains
                      # finish, the PE streams chunk B's matmuls
                      for srcs, _, _, dr1, _, _ in plans:
                          layer(srcs[ev], w1s, dr1)
                      for _, d1s, _, _, dr2, _ in plans:
                          layer(d1s[ev], w2s, dr2)
                      for _, _, d2s, _, _, dr3 in plans:
                          layer(d2s[ev], w3s, dr3)

        if reps == 1:
            steps_body()
        else:
            # timing mode: repeat the whole integration on-device so
            # kernel time dwarfs the host/RPC dispatch noise
            with tc.For_i(0, reps, 1):
                steps_body()

        # ---- store all chunks, transposed back ----
        for c in range(chunks):
            rows0 = c * NCHUNK
            for bt in range(NCHUNK // P):
                stg = stage_pool.tile([P, HID], F32, tag="stg")
                for j in range(MT):
                    pt = pstr.tile([P, P], F32, tag="pt")
                    nc.tensor.transpose(pt[:], hT[c][:, j, P * bt : P * (bt + 1)], idt[:])
                    nc.vector.tensor_copy(stg[:, P * j : P * (j + 1)], pt[:])
                dma(out=out[rows0 + P * bt : rows0 + P * (bt + 1), :], in_=stg[:])

    nc.compile()
    return nc


_NC_CACHE = {}


def get_nc(steps=STEPS, chunks=CHUNKS, reps=1, mode="fp8", paired=True):
    key = (steps, chunks, reps, mode, paired)
    if key not in _NC_CACHE:
        _NC_CACHE[key] = build_nc(steps, chunks, reps, mode, paired)
    return _NC_CACHE[key]


def make_in_maps(inputs, mode="fp8"):
    eye = np.eye(P, dtype=np.float32)
    full = {k: np.ascontiguousarray(np.asarray(v, dtype=np.float32))
            for k, v in inputs.items()}
    full["w1row"] = np.ascontiguousarray(full["W1"][HID])
    full["W1"] = full["W1"][:HID]
    for w in ("W1", "W2", "W3"):
        if mode == "fp8":
            q = np.clip(full[w] * SW, -240.0, 240.0)
            full[w] = np.ascontiguousarray(q.astype(ml_dtypes.float8_e4m3))
        else:
            full[w] = np.ascontiguousarray(full[w].astype(np.float16))
    in_maps = []
    for c in range(N_CORES):
        m = dict(full)
        m["h"] = np.ascontiguousarray(
            full["h"][c * CORE_BATCH : (c + 1) * CORE_BATCH]
        )
        m["ident"] = eye
        in_maps.append(m)
    return in_maps


def kernel(**inputs):
    nc = get_nc()
    in_maps = make_in_maps(inputs)
    res = run_bass_kernel_spmd(nc, in_maps, list(range(N_CORES)))
    return np.concatenate(
        [res.results[c]["out"] for c in range(N_CORES)], axis=0
    )
